# revision 24
# baseline (speedup 1.0000x reference)
"""BiologicallyInformedAttention TRN2 kernel v2 (8 NeuronCores, axon/PJRT).

Sharding: B*H = 32 (batch, head) pairs over 8 cores -> core c handles batch
c//2, heads (c%2)*4 .. +4 (= 2 head-pairs). Projection weights column-sliced
per core; x transposed host-side so matmuls contract over the partition dim.

v2 design (single flat tile stream, ACT-exp is the roofline engine):
  - scores via fp8e4 DoubleRow matmuls at 0.5 cyc/col: q stored hi/lo
    (q_hi=fp8(q), q_lo=fp8(q-q_hi)) in the two DoubleRow k-tile slots,
    k single fp8 (stationary broadcast_to over the slot dim). Halves the
    score PE time vs f32r; rel err ~4e-3 (vs 2e-2 gate).
  - prior (+8*pw*I pre-scale) accumulated into the score PSUM group by a
    bf16 identity matmul on the PE (removes DVE prior adds).
  - exp on ACT (scale=0.125 fused); optionally KPOLY tiles/phase evaluated
    as a normalized cubic on the DVE ((((u+c1)u+c2)u+c3)*c4 ~ exp(u/8),
    3 instrs) to offload the ACT bottleneck.
  - AV f32r (fp8 fails precision: attended is a 1/sqrt(S)-scale mean).
    Row 64 of v_aug (ones) gives softmax denominators for free.
  - normalize: reciprocal_approx_fast direct from AV PSUM row, gpsimd
    partition_broadcast, DVE mul from PSUM -> attnT (no copies).
  - 8 phases/pass of (pair, head, qh) x 16 kt tiles; one [66,1024] AV psum
    accumulator live at a time; score psum 3-deep so ACT never waits.
  - projection pieces + next-pass pieces are woven into the tile stream as
    'extras' so the PE fills its slack and ACT starts immediately; v_aug is
    double-buffered across passes to decouple the weave.
Host: out[b] = (outT[2b] + outT[2b+1]).T + bo.
"""
import os
import numpy as np
from contextlib import ExitStack

import concourse.bacc as bacc
import concourse.tile as tile
from concourse import mybir
from concourse.bass_utils import run_bass_kernel_spmd

B, S, D, H, DH = 4, 2048, 512, 8, 64
HPC = H // 2          # heads per core = 4 (2 pairs)
W_COLS = HPC * DH     # 256 per-core projection columns
N_CORES = 8

f32 = mybir.dt.float32
f32r = mybir.dt.float32r
fp8 = mybir.dt.float8e4
bf16 = mybir.dt.bfloat16
AF = mybir.ActivationFunctionType
ALU = mybir.AluOpType
DR = mybir.MatmulPerfMode.DoubleRow

# (((u+C1)*u + C2)*u + C3)*C4 ~= exp(u/8), minimax rel err 1.2% over |u|<=10
C1, C2, C3, C4 = 28.990146918350636, 426.06823629990475, 3302.555526494704, \
    3.008438007849662e-4

_BUILT = {}


def _poly_tiles():
    """Set of (phase_idx, kt) evaluated with the DVE cubic instead of ACT exp.
    KPOLY = tiles per phase (0..16)."""
    n = int(os.environ.get("KPOLY", "0"))
    # spread mid-phase, avoid kt 0/15 (phase boundaries)
    slots = [5, 11, 8, 2, 13, 6, 9, 3, 12, 7, 10, 4, 14, 1, 15, 0]
    return {(ph, kt) for ph in range(8) for kt in slots[:n]}


def _build(repeat=1):
    nc = bacc.Bacc("TRN2", target_bir_lowering=False)

    xT_d = nc.declare_dram_parameter("xT", [D, S], f32r, isOutput=False)
    wq_d = nc.declare_dram_parameter("wq", [D, W_COLS], f32r, isOutput=False)
    wk_d = nc.declare_dram_parameter("wk", [D, W_COLS], f32r, isOutput=False)
    wv_d = nc.declare_dram_parameter("wv", [D, W_COLS], f32r, isOutput=False)
    wo_d = nc.declare_dram_parameter("wo", [W_COLS, DH], f32r, isOutput=False)
    bq_d = nc.declare_dram_parameter("bq", [128, 2], f32, isOutput=False)
    bk_d = nc.declare_dram_parameter("bk", [128, 2], f32, isOutput=False)
    bv_d = nc.declare_dram_parameter("bv", [1, W_COLS], f32r, isOutput=False)
    ones_row_d = nc.declare_dram_parameter("ones_row", [1, 128], f32r, isOutput=False)
    ones_blk_d = nc.declare_dram_parameter("ones_blk", [128, 8], f32r, isOutput=False)
    pw8i_d = nc.declare_dram_parameter("pw8i", [128, 128], bf16, isOutput=False)
    eye_d = nc.declare_dram_parameter("eye", [128, 128], bf16, isOutput=False)
    pcoef_d = nc.declare_dram_parameter("pcoef", [128, 4], f32, isOutput=False)
    pw8f_d = nc.declare_dram_parameter("pw8f", [128, 128], f32, isOutput=False)
    outT_d = nc.declare_dram_parameter("outT", [DH, S], f32, isOutput=True)

    poly = _poly_tiles()
    POLY_ENG = os.environ.get("KPOLYENG", "pool")
    PRIOR_ENG = os.environ.get("KPRIOR", "pe")
    sc_bufs = int(os.environ.get("KSCBUFS", "3"))
    et_bufs = int(os.environ.get("KETBUFS", "10"))
    av_delay = int(os.environ.get("KAVDELAY", "6"))

    with tile.TileContext(nc) as tc, ExitStack() as ctx:
        cp = ctx.enter_context(tc.tile_pool(name="cp", bufs=1))

        # ---------- persistent tiles ----------
        xr = [cp.tile([128, S], f32r, tag=f"xr{i}", name=f"xr{i}") for i in range(4)]
        wqr = [cp.tile([128, W_COLS], f32r, tag=f"wqr{i}", name=f"wqr{i}") for i in range(4)]
        wkr = [cp.tile([128, W_COLS], f32r, tag=f"wkr{i}", name=f"wkr{i}") for i in range(4)]
        wvr = [cp.tile([128, W_COLS], f32r, tag=f"wvr{i}", name=f"wvr{i}") for i in range(4)]
        wor = cp.tile([DH, W_COLS], f32r, tag="wor", name="wor")
        bq_t = cp.tile([128, 2], f32, tag="bq", name="bq")
        bk_t = cp.tile([128, 2], f32, tag="bk", name="bk")
        bvr = cp.tile([1, W_COLS], f32r, tag="bvr", name="bvr")
        ones_col = cp.tile([1, 128], f32r, tag="ones_col", name="ones_col")
        ones_blk = cp.tile([128, 8], f32r, tag="ones_blk", name="ones_blk")
        pw8i = cp.tile([128, 128], bf16, tag="pw8i", name="pw8i")
        eye_t = cp.tile([128, 128], bf16, tag="eye", name="eye")
        pcoef = cp.tile([128, 4], f32, tag="pcoef", name="pcoef")
        pw8f = cp.tile([128, 128], f32, tag="pw8f", name="pw8f")
        q8 = [cp.tile([128, 2, S], fp8, tag=f"q8{p}", name=f"q8{p}") for p in range(2)]
        k8 = [cp.tile([128, S], fp8, tag=f"k8{p}", name=f"k8{p}") for p in range(2)]
        v_aug = [[cp.tile([128, HPC * 66], f32r, tag=f"va{s}_{st}", name=f"va{s}_{st}")
                  for st in range(16)] for s in range(2)]
        attnT = [cp.tile([DH, S], f32r, tag=f"at{h}", name=f"at{h}") for h in range(HPC)]
        outT_s = cp.tile([DH, S], f32, tag="outT", name="outT")

        # ---------- loads (once) ----------
        for di in range(4):
            nc.sync.dma_start(wqr[di][:], wq_d[di * 128:(di + 1) * 128, :])
            nc.sync.dma_start(wkr[di][:], wk_d[di * 128:(di + 1) * 128, :])
        for sc4 in range(4):
            s0 = sc4 * 512
            for di in range(4):
                nc.sync.dma_start(xr[di][:, s0:s0 + 512],
                                  xT_d[di * 128:(di + 1) * 128, s0:s0 + 512])
            if sc4 == 1:
                for di in range(4):
                    nc.sync.dma_start(wvr[di][:], wv_d[di * 128:(di + 1) * 128, :])
        for h in range(HPC):
            nc.sync.dma_start(wor[:, h * DH:(h + 1) * DH],
                              wo_d[h * DH:(h + 1) * DH, :])
        nc.sync.dma_start(bq_t[:], bq_d[:])
        nc.sync.dma_start(bk_t[:], bk_d[:])
        nc.sync.dma_start(bvr[:], bv_d[:])
        nc.sync.dma_start(ones_col[:], ones_row_d[:])
        nc.sync.dma_start(ones_blk[:], ones_blk_d[:])
        nc.sync.dma_start(pw8i[:], pw8i_d[:])
        nc.sync.dma_start(eye_t[:], eye_d[:])
        nc.sync.dma_start(pcoef[:], pcoef_d[:])
        nc.sync.dma_start(pw8f[:], pw8f_d[:])

        with tc.tile_pool(name="scp", bufs=sc_bufs, space="PSUM") as scp, \
             tc.tile_pool(name="avp", bufs=1, space="PSUM") as avp, \
             tc.tile_pool(name="etp", bufs=et_bufs) as etp, \
             tc.tile_pool(name="ptp", bufs=1) as ptp, \
             tc.tile_pool(name="nrm", bufs=1) as nrm:

            # one-time: ones columns of both v_aug sets
            for vset in range(2):
                for st in range(16):
                    va = v_aug[vset][st][:].rearrange("p (h c) -> p h c", c=66)
                    nc.vector.tensor_copy(
                        va[:, :, DH:66],
                        ones_blk[:].rearrange("p (h c) -> p h c", c=2))

            # ---------- pieces ----------
            def qk_piece(p, sc4, which):
                s0 = sc4 * 512
                pt = scp.tile([128, 512], f32, tag="sc", name="pj")
                wr = wqr if which == 'q' else wkr
                for di in range(4):
                    nc.tensor.matmul(pt[:], wr[di][:, p * 128:(p + 1) * 128],
                                     xr[di][:, s0:s0 + 512],
                                     start=(di == 0), stop=(di == 3))
                if which == 'q':
                    qv = q8[p][:]
                    nc.vector.tensor_scalar_add(
                        qv[:, 0, s0:s0 + 512], pt[:], bq_t[:, p:p + 1])
                    nc.vector.scalar_tensor_tensor(
                        qv[:, 1, s0:s0 + 512], pt[:], bq_t[:, p:p + 1],
                        qv[:, 0, s0:s0 + 512], ALU.add, ALU.subtract)
                else:
                    nc.vector.tensor_scalar_add(
                        k8[p][:, s0:s0 + 512], pt[:], bk_t[:, p:p + 1])

            def v_piece(vset, st):
                pv = scp.tile([128, W_COLS], f32, tag="sc", name="pv")
                for di in range(4):
                    nc.tensor.matmul(pv[:],
                                     xr[di][:, st * 128:(st + 1) * 128],
                                     wvr[di][:],
                                     start=(di == 0), stop=False)
                nc.tensor.matmul(pv[:], ones_col[:], bvr[:],
                                 start=False, stop=True)
                va = v_aug[vset][st][:].rearrange("p (h c) -> p h c", c=66)
                nc.vector.tensor_copy(
                    va[:, :, 0:DH],
                    pv[:].rearrange("p (h c) -> p h c", c=DH))

            def out_proj(qh, rep):
                for sc4 in (2 * qh, 2 * qh + 1):
                    s0 = sc4 * 512
                    po = scp.tile([DH, 512], f32, tag="sc", name="po")
                    for h in range(HPC):
                        nc.tensor.matmul(po[:],
                                         wor[:, h * DH:(h + 1) * DH],
                                         attnT[h][:, s0:s0 + 512],
                                         start=(h == 0), stop=(h == HPC - 1))
                    nc.vector.tensor_copy(outT_s[:, s0:s0 + 512], po[:])
                    nc.sync.dma_start(outT_d[:, s0:s0 + 512],
                                      outT_s[:, s0:s0 + 512])

            # ---------- flat tile stream ----------
            phases = [(0, 0, 0), (0, 1, 0), (0, 0, 1), (0, 1, 1),
                      (1, 0, 0), (1, 1, 0), (1, 0, 1), (1, 1, 1)]

            def emit_scores(p, hl, qh, kt):
                base = hl * 64
                q0 = qh * 1024
                k0 = kt * 128
                off = k0 - q0
                sc = scp.tile([128, 1024], f32, tag="sc", name="sc")
                lhs = k8[p][base:base + 64, k0:k0 + 128] \
                    .unsqueeze(1).broadcast_to([64, 2, 128])
                qv = q8[p][:]
                for reg in range(2):           # 512-col chunks = zero regions
                    r0 = reg * 512
                    has_diag = 0 <= off - r0 < 512
                    pe_prior = has_diag and PRIOR_ENG == "pe"
                    nc.tensor.matmul(
                        sc[:, r0:r0 + 512], lhs,
                        qv[base:base + 64, :, q0 + r0:q0 + r0 + 512],
                        start=True, stop=not pe_prior, perf_mode=DR)
                    if pe_prior:
                        nc.tensor.matmul(sc[:, off:off + 128], eye_t[:],
                                         pw8i[:], start=False, stop=True)
                    elif has_diag:
                        nc.vector.tensor_add(sc[:, off:off + 128],
                                             sc[:, off:off + 128], pw8f[:])
                return sc

            def emit_exp(sc, ph, kt):
                # cubic in u = c*s with c = C4^(1/3):
                # et = ((u + C1*c)*u + C2*c^2)*u + C3*c^3  ==  P3(s)*C4 ~ exp(s/8)
                et = etp.tile([128, 1024], f32r, tag="et", name="et")
                if (ph, kt) in poly:
                    eng = nc.gpsimd if POLY_ENG == "pool" else nc.vector
                    u = ptp.tile([128, 1024], f32, tag="pu", name="pu")
                    t1 = ptp.tile([128, 1024], f32, tag="pt1", name="pt1")
                    eng.tensor_scalar_mul(u[:], sc[:], pcoef[:, 0:1])
                    eng.scalar_tensor_tensor(
                        t1[:], u[:], pcoef[:, 1:2], u[:], ALU.add, ALU.mult)
                    eng.scalar_tensor_tensor(
                        t1[:], t1[:], pcoef[:, 2:3], u[:], ALU.add, ALU.mult)
                    eng.tensor_scalar_add(et[:], t1[:], pcoef[:, 3:4])
                else:
                    nc.scalar.activation(et[:], sc[:], AF.Exp, scale=0.125)
                return et

            def emit_av(av, vset, p, hl, kt, et, start, stop):
                hcol = (2 * p + hl) * 66
                for qc in range(2):
                    qq = qc * 512
                    nc.tensor.matmul(
                        av[:, qq:qq + 512],
                        v_aug[vset][kt][:, hcol:hcol + 66],
                        et[:, qq:qq + 512],
                        start=start, stop=stop)

            def emit_norm(av, p, hl, qh):
                hh = 2 * p + hl
                q0 = qh * 1024
                sums = nrm.tile([1, 1024], f32, tag="sums", name="sums", bufs=1)
                nc.vector.tensor_copy(sums[:], av[DH:DH + 1, :])
                recip = nrm.tile([1, 1024], f32, tag="recip", name="recip", bufs=1)
                nc.vector.reciprocal_approx_fast(recip[:], sums[:])
                rB = nrm.tile([DH, 1024], f32, tag="rB", name="rB", bufs=2)
                nc.gpsimd.partition_broadcast(rB[:], recip[:])
                nc.vector.tensor_mul(attnT[hh][:, q0:q0 + 1024],
                                     av[0:DH, :], rB[:])

            # extras: global slot -> list of closures (one rep's stream = 128)
            def build_extras(rep):
                ex = {}
                vset_next = (rep + 1) % 2
                def add(slot, fn):
                    ex.setdefault(slot, []).append(fn)
                # pair-1 q/k proj pieces in phases 1-2 (needed by phase 5)
                for i in range(4):
                    add(2 + 8 * i, lambda sc4=i: qk_piece(1, sc4, 'k'))
                    add(6 + 8 * i, lambda sc4=i: qk_piece(1, sc4, 'q'))
                if rep + 1 < repeat:
                    # next pass: v pieces phases 3-5, pair-0 q/k phases 6-7
                    for st in range(16):
                        add(33 + 3 * st, lambda st=st: v_piece(vset_next, st))
                    for i in range(4):
                        add(82 + 8 * i, lambda sc4=i: qk_piece(0, sc4, 'k'))
                        add(86 + 8 * i, lambda sc4=i: qk_piece(0, sc4, 'q'))
                return ex

            # prologue (pass 0 inputs)
            for sc4 in range(4):
                qk_piece(0, sc4, 'k')
                qk_piece(0, sc4, 'q')
            for st in range(16):
                v_piece(0, st)

            def drain_one(pending, rep):
                av, vset, p, hl, qh, kt, et, ph = pending.pop(0)
                emit_av(av, vset, p, hl, kt, et, kt == 0, kt == 15)
                if kt == 15:
                    emit_norm(av, p, hl, qh)
                    if ph == 5:
                        out_proj(0, rep)
                    elif ph == 7:
                        out_proj(1, rep)

            for rep in range(repeat):
                vset = rep % 2
                extras = build_extras(rep)
                pending = []
                av = None
                for slot in range(128):
                    ph, kt = divmod(slot, 16)
                    p, hl, qh = phases[ph]
                    if kt == 0:
                        av = avp.tile([66, 1024], f32, tag="av", name="av")
                    sc = emit_scores(p, hl, qh, kt)
                    et = emit_exp(sc, ph, kt)
                    pending.append((av, vset, p, hl, qh, kt, et, ph))
                    while len(pending) > av_delay:
                        drain_one(pending, rep)
                    for fn in extras.pop(slot, ()):
                        fn()
                while pending:
                    drain_one(pending, rep)

    nc.finalize()
    return nc


def _get_nc(repeat=1):
    key = (repeat, os.environ.get("KPOLY", "0"), os.environ.get("KSCBUFS", "3"),
           os.environ.get("KETBUFS", "10"), os.environ.get("KAVDELAY", "6"),
           os.environ.get("KPRIOR", "pe"), os.environ.get("KPOLYENG", "pool"))
    if key not in _BUILT:
        _BUILT[key] = _build(repeat)
    return _BUILT[key]


def _make_in_maps(x, Wq, bq, Wk, bk, Wv, bv, Wo, bo, prior_weight):
    pw8i = (8.0 * float(prior_weight[0])) * np.eye(128, dtype=np.float32)
    eye = np.eye(128, dtype=np.float32)
    bf = mybir.dt.np(bf16)
    ones_row = np.ones((1, 128), np.float32)
    ones_blk = np.ones((128, 8), np.float32)
    c = C4 ** (1.0 / 3.0)
    pcoef_row = np.tile(np.array([c, C1 * c, C2 * c * c, C3 * c ** 3],
                                 np.float32), (128, 1))
    xT = [np.ascontiguousarray(x[b].T) for b in range(B)]
    in_maps = []
    for c in range(N_CORES):
        b, half = c // 2, c % 2
        cs = slice(half * W_COLS, (half + 1) * W_COLS)
        in_maps.append({
            "xT": xT[b],
            "wq": np.ascontiguousarray(Wq[:, cs]),
            "wk": np.ascontiguousarray(Wk[:, cs]),
            "wv": np.ascontiguousarray(Wv[:, cs]),
            "wo": np.ascontiguousarray(Wo[cs, :]),
            "bq": np.ascontiguousarray(bq[cs].reshape(2, 128).T),
            "bk": np.ascontiguousarray(bk[cs].reshape(2, 128).T),
            "bv": np.ascontiguousarray(bv[cs].reshape(1, W_COLS)),
            "pw8i": pw8i.astype(bf),
            "eye": eye.astype(bf),
            "pcoef": pcoef_row,
            "pw8f": pw8i,
            "ones_row": ones_row,
            "ones_blk": ones_blk,
        })
    return in_maps


def run(inputs, trace=False, trace_cores=None):
    """Execute on 8 cores; returns (output [B,S,DH] f32, BassKernelResults)."""
    args = {k: np.asarray(v) for k, v in inputs.items()}
    nc = _get_nc()
    in_maps = _make_in_maps(
        args["x"], args["Wq"], args["bq"], args["Wk"], args["bk"],
        args["Wv"], args["bv"], args["Wo"], args["bo"], args["prior_weight"])
    res = run_bass_kernel_spmd(
        nc, in_maps, list(range(N_CORES)), trace=trace,
        **({"trace_cores": trace_cores} if trace_cores else {}))
    bo = args["bo"].astype(np.float32)
    out = np.empty((B, S, DH), np.float32)
    for b in range(B):
        acc = res.results[2 * b]["outT"] + res.results[2 * b + 1]["outT"]
        out[b] = acc.T + bo
    return out, res


def kernel(**inputs) -> np.ndarray:
    out, _ = run(inputs, trace=False)
    return out


# revision 27
# speedup vs baseline: 2.6642x; 2.6642x over previous
"""BiologicallyInformedAttention TRN2 kernel v2 (8 NeuronCores, axon/PJRT).

Sharding: B*H = 32 (batch, head) pairs over 8 cores -> core c handles batch
c//2, heads (c%2)*4 .. +4 (= 2 head-pairs). Projection weights column-sliced
per core; x transposed host-side so matmuls contract over the partition dim.

v2 design (single flat tile stream, ACT-exp is the roofline engine):
  - scores via fp8e4 DoubleRow matmuls at 0.5 cyc/col: q stored hi/lo
    (q_hi=fp8(q), q_lo=fp8(q-q_hi)) in the two DoubleRow k-tile slots,
    k single fp8 (stationary broadcast_to over the slot dim). Halves the
    score PE time vs f32r; rel err ~4e-3 (vs 2e-2 gate).
  - prior (+8*pw*I pre-scale) accumulated into the score PSUM group by a
    bf16 identity matmul on the PE (removes DVE prior adds).
  - exp on ACT (scale=0.125 fused); optionally KPOLY tiles/phase evaluated
    as a normalized cubic on the DVE ((((u+c1)u+c2)u+c3)*c4 ~ exp(u/8),
    3 instrs) to offload the ACT bottleneck.
  - AV f32r (fp8 fails precision: attended is a 1/sqrt(S)-scale mean).
    Row 64 of v_aug (ones) gives softmax denominators for free.
  - normalize: reciprocal_approx_fast direct from AV PSUM row, gpsimd
    partition_broadcast, DVE mul from PSUM -> attnT (no copies).
  - 8 phases/pass of (pair, head, qh) x 16 kt tiles; one [66,1024] AV psum
    accumulator live at a time; score psum 3-deep so ACT never waits.
  - projection pieces + next-pass pieces are woven into the tile stream as
    'extras' so the PE fills its slack and ACT starts immediately; v_aug is
    double-buffered across passes to decouple the weave.
Host: out[b] = (outT[2b] + outT[2b+1]).T + bo.
"""
import os
import numpy as np
from contextlib import ExitStack

import concourse.bacc as bacc
import concourse.tile as tile
from concourse import mybir
from concourse.bass_utils import run_bass_kernel_spmd

B, S, D, H, DH = 4, 2048, 512, 8, 64
HPC = H // 2          # heads per core = 4 (2 pairs)
W_COLS = HPC * DH     # 256 per-core projection columns
N_CORES = 8

f32 = mybir.dt.float32
f32r = mybir.dt.float32r
fp8 = mybir.dt.float8e4
bf16 = mybir.dt.bfloat16
AF = mybir.ActivationFunctionType
ALU = mybir.AluOpType
DR = mybir.MatmulPerfMode.DoubleRow

# (((u+C1)*u + C2)*u + C3)*C4 ~= exp(u/8), minimax rel err 1.2% over |u|<=10
C1, C2, C3, C4 = 28.990146918350636, 426.06823629990475, 3302.555526494704, \
    3.008438007849662e-4

_BUILT = {}


def _poly_tiles():
    """Set of (phase_idx, kt) evaluated with the DVE cubic instead of ACT exp.
    KPOLY = tiles per phase (0..16)."""
    n = int(os.environ.get("KPOLY", "0"))
    # spread mid-phase, avoid kt 0/15 (phase boundaries)
    slots = [5, 11, 8, 2, 13, 6, 9, 3, 12, 7, 10, 4, 14, 1, 15, 0]
    return {(ph, kt) for ph in range(8) for kt in slots[:n]}


def _build(repeat=1):
    nc = bacc.Bacc("TRN2", target_bir_lowering=False)

    xT_d = nc.declare_dram_parameter("xT", [D, S], f32r, isOutput=False)
    wq_d = nc.declare_dram_parameter("wq", [D, W_COLS], f32r, isOutput=False)
    wk_d = nc.declare_dram_parameter("wk", [D, W_COLS], f32r, isOutput=False)
    wv_d = nc.declare_dram_parameter("wv", [D, W_COLS], f32r, isOutput=False)
    wo_d = nc.declare_dram_parameter("wo", [W_COLS, DH], f32r, isOutput=False)
    bq_d = nc.declare_dram_parameter("bq", [128, 2], f32, isOutput=False)
    bk_d = nc.declare_dram_parameter("bk", [128, 2], f32, isOutput=False)
    bv_d = nc.declare_dram_parameter("bv", [1, W_COLS], f32r, isOutput=False)
    ones_row_d = nc.declare_dram_parameter("ones_row", [1, 128], f32r, isOutput=False)
    ones_blk_d = nc.declare_dram_parameter("ones_blk", [128, 8], f32r, isOutput=False)
    pw8i_d = nc.declare_dram_parameter("pw8i", [128, 128], bf16, isOutput=False)
    eye_d = nc.declare_dram_parameter("eye", [128, 128], bf16, isOutput=False)
    pcoef_d = nc.declare_dram_parameter("pcoef", [128, 4], f32, isOutput=False)
    pw8f_d = nc.declare_dram_parameter("pw8f", [128, 128], f32, isOutput=False)
    outT_d = nc.declare_dram_parameter("outT", [DH, S], f32, isOutput=True)

    poly = _poly_tiles()
    POLY_ENG = os.environ.get("KPOLYENG", "pool")
    PRIOR_ENG = os.environ.get("KPRIOR", "pe")
    NORM_ENG = os.environ.get("KNORMENG", "dve")
    sc_bufs = int(os.environ.get("KSCBUFS", "3"))
    et_bufs = int(os.environ.get("KETBUFS", "10"))
    av_delay = int(os.environ.get("KAVDELAY", "6"))

    with tile.TileContext(nc) as tc, ExitStack() as ctx:
        cp = ctx.enter_context(tc.tile_pool(name="cp", bufs=1))

        # ---------- persistent tiles ----------
        xr = [cp.tile([128, S], f32r, tag=f"xr{i}", name=f"xr{i}") for i in range(4)]
        wqr = [cp.tile([128, W_COLS], f32r, tag=f"wqr{i}", name=f"wqr{i}") for i in range(4)]
        wkr = [cp.tile([128, W_COLS], f32r, tag=f"wkr{i}", name=f"wkr{i}") for i in range(4)]
        wvr = [cp.tile([128, W_COLS], f32r, tag=f"wvr{i}", name=f"wvr{i}") for i in range(4)]
        wor = cp.tile([DH, W_COLS], f32r, tag="wor", name="wor")
        bq_t = cp.tile([128, 2], f32, tag="bq", name="bq")
        bk_t = cp.tile([128, 2], f32, tag="bk", name="bk")
        bvr = cp.tile([1, W_COLS], f32r, tag="bvr", name="bvr")
        ones_col = cp.tile([1, 128], f32r, tag="ones_col", name="ones_col")
        ones_blk = cp.tile([128, 8], f32r, tag="ones_blk", name="ones_blk")
        pw8i = cp.tile([128, 128], bf16, tag="pw8i", name="pw8i")
        eye_t = cp.tile([128, 128], bf16, tag="eye", name="eye")
        pcoef = cp.tile([128, 4], f32, tag="pcoef", name="pcoef")
        pw8f = cp.tile([128, 128], f32, tag="pw8f", name="pw8f")
        q8 = [cp.tile([128, 2, S], fp8, tag=f"q8{p}", name=f"q8{p}") for p in range(2)]
        k8 = [cp.tile([128, S], fp8, tag=f"k8{p}", name=f"k8{p}") for p in range(2)]
        v_aug = [[cp.tile([128, HPC * 66], f32r, tag=f"va{s}_{st}", name=f"va{s}_{st}")
                  for st in range(16)] for s in range(2)]
        attnT = [cp.tile([DH, S], f32r, tag=f"at{h}", name=f"at{h}") for h in range(HPC)]
        outT_s = cp.tile([DH, S], f32, tag="outT", name="outT")

        # ---------- loads (once) ----------
        for di in range(4):
            nc.sync.dma_start(wqr[di][:], wq_d[di * 128:(di + 1) * 128, :])
            nc.sync.dma_start(wkr[di][:], wk_d[di * 128:(di + 1) * 128, :])
        for sc4 in range(4):
            s0 = sc4 * 512
            for di in range(4):
                nc.sync.dma_start(xr[di][:, s0:s0 + 512],
                                  xT_d[di * 128:(di + 1) * 128, s0:s0 + 512])
            if sc4 == 1:
                for di in range(4):
                    nc.sync.dma_start(wvr[di][:], wv_d[di * 128:(di + 1) * 128, :])
        for h in range(HPC):
            nc.sync.dma_start(wor[:, h * DH:(h + 1) * DH],
                              wo_d[h * DH:(h + 1) * DH, :])
        nc.sync.dma_start(bq_t[:], bq_d[:])
        nc.sync.dma_start(bk_t[:], bk_d[:])
        nc.sync.dma_start(bvr[:], bv_d[:])
        nc.sync.dma_start(ones_col[:], ones_row_d[:])
        nc.sync.dma_start(ones_blk[:], ones_blk_d[:])
        nc.sync.dma_start(pw8i[:], pw8i_d[:])
        nc.sync.dma_start(eye_t[:], eye_d[:])
        nc.sync.dma_start(pcoef[:], pcoef_d[:])
        nc.sync.dma_start(pw8f[:], pw8f_d[:])

        with tc.tile_pool(name="scp", bufs=sc_bufs, space="PSUM") as scp, \
             tc.tile_pool(name="avp", bufs=1, space="PSUM") as avp, \
             tc.tile_pool(name="etp", bufs=et_bufs) as etp, \
             tc.tile_pool(name="ptp", bufs=1) as ptp, \
             tc.tile_pool(name="nrm", bufs=1) as nrm:

            # one-time: ones columns of both v_aug sets
            for vset in range(2):
                for st in range(16):
                    va = v_aug[vset][st][:].rearrange("p (h c) -> p h c", c=66)
                    nc.vector.tensor_copy(
                        va[:, :, DH:66],
                        ones_blk[:].rearrange("p (h c) -> p h c", c=2))

            # ---------- pieces ----------
            def qk_piece(p, sc4, which):
                s0 = sc4 * 512
                pt = scp.tile([128, 512], f32, tag="sc", name="pj")
                wr = wqr if which == 'q' else wkr
                for di in range(4):
                    nc.tensor.matmul(pt[:], wr[di][:, p * 128:(p + 1) * 128],
                                     xr[di][:, s0:s0 + 512],
                                     start=(di == 0), stop=(di == 3))
                if which == 'q':
                    qv = q8[p][:]
                    nc.vector.tensor_scalar_add(
                        qv[:, 0, s0:s0 + 512], pt[:], bq_t[:, p:p + 1])
                    nc.vector.scalar_tensor_tensor(
                        qv[:, 1, s0:s0 + 512], pt[:], bq_t[:, p:p + 1],
                        qv[:, 0, s0:s0 + 512], ALU.add, ALU.subtract)
                else:
                    nc.vector.tensor_scalar_add(
                        k8[p][:, s0:s0 + 512], pt[:], bk_t[:, p:p + 1])

            def v_piece(vset, st):
                pv = scp.tile([128, W_COLS], f32, tag="sc", name="pv")
                for di in range(4):
                    nc.tensor.matmul(pv[:],
                                     xr[di][:, st * 128:(st + 1) * 128],
                                     wvr[di][:],
                                     start=(di == 0), stop=False)
                nc.tensor.matmul(pv[:], ones_col[:], bvr[:],
                                 start=False, stop=True)
                va = v_aug[vset][st][:].rearrange("p (h c) -> p h c", c=66)
                nc.vector.tensor_copy(
                    va[:, :, 0:DH],
                    pv[:].rearrange("p (h c) -> p h c", c=DH))

            def out_proj(qh, rep):
                for sc4 in (2 * qh, 2 * qh + 1):
                    s0 = sc4 * 512
                    po = scp.tile([DH, 512], f32, tag="sc", name="po")
                    for h in range(HPC):
                        nc.tensor.matmul(po[:],
                                         wor[:, h * DH:(h + 1) * DH],
                                         attnT[h][:, s0:s0 + 512],
                                         start=(h == 0), stop=(h == HPC - 1))
                    nc.vector.tensor_copy(outT_s[:, s0:s0 + 512], po[:])
                    nc.sync.dma_start(outT_d[:, s0:s0 + 512],
                                      outT_s[:, s0:s0 + 512])

            # ---------- flat tile stream ----------
            phases = [(0, 0, 0), (0, 1, 0), (0, 0, 1), (0, 1, 1),
                      (1, 0, 0), (1, 1, 0), (1, 0, 1), (1, 1, 1)]

            def emit_scores(p, hl, qh, kt):
                base = hl * 64
                q0 = qh * 1024
                k0 = kt * 128
                off = k0 - q0
                sc = scp.tile([128, 1024], f32, tag="sc", name="sc")
                lhs = k8[p][base:base + 64, k0:k0 + 128] \
                    .unsqueeze(1).broadcast_to([64, 2, 128])
                qv = q8[p][:]
                for reg in range(2):           # 512-col chunks = zero regions
                    r0 = reg * 512
                    has_diag = 0 <= off - r0 < 512
                    pe_prior = has_diag and PRIOR_ENG == "pe"
                    nc.tensor.matmul(
                        sc[:, r0:r0 + 512], lhs,
                        qv[base:base + 64, :, q0 + r0:q0 + r0 + 512],
                        start=True, stop=not pe_prior, perf_mode=DR)
                    if pe_prior:
                        nc.tensor.matmul(sc[:, off:off + 128], eye_t[:],
                                         pw8i[:], start=False, stop=True)
                    elif has_diag:
                        nc.vector.tensor_add(sc[:, off:off + 128],
                                             sc[:, off:off + 128], pw8f[:])
                return sc

            def emit_exp(sc, ph, kt):
                # cubic in u = c*s with c = C4^(1/3):
                # et = ((u + C1*c)*u + C2*c^2)*u + C3*c^3  ==  P3(s)*C4 ~ exp(s/8)
                et = etp.tile([128, 1024], f32r, tag="et", name="et")
                if (ph, kt) in poly:
                    eng = nc.gpsimd if POLY_ENG == "pool" else nc.vector
                    u = ptp.tile([128, 1024], f32, tag="pu", name="pu")
                    t1 = ptp.tile([128, 1024], f32, tag="pt1", name="pt1")
                    eng.tensor_scalar_mul(u[:], sc[:], pcoef[:, 0:1])
                    eng.scalar_tensor_tensor(
                        t1[:], u[:], pcoef[:, 1:2], u[:], ALU.add, ALU.mult)
                    eng.scalar_tensor_tensor(
                        t1[:], t1[:], pcoef[:, 2:3], u[:], ALU.add, ALU.mult)
                    eng.tensor_scalar_add(et[:], t1[:], pcoef[:, 3:4])
                else:
                    nc.scalar.activation(et[:], sc[:], AF.Exp, scale=0.125)
                return et

            def emit_av(av, vset, p, hl, kt, et, start, stop):
                hcol = (2 * p + hl) * 66
                for qc in range(2):
                    qq = qc * 512
                    nc.tensor.matmul(
                        av[:, qq:qq + 512],
                        v_aug[vset][kt][:, hcol:hcol + 66],
                        et[:, qq:qq + 512],
                        start=start, stop=stop)

            def emit_norm(av, p, hl, qh):
                hh = 2 * p + hl
                q0 = qh * 1024
                sums = nrm.tile([1, 1024], f32, tag="sums", name="sums", bufs=1)
                nc.vector.tensor_copy(sums[:], av[DH:DH + 1, :])
                recip = nrm.tile([1, 1024], f32, tag="recip", name="recip", bufs=1)
                nc.vector.reciprocal_approx_fast(recip[:], sums[:])
                rB = nrm.tile([DH, 1024], f32, tag="rB", name="rB", bufs=2)
                nc.gpsimd.partition_broadcast(rB[:], recip[:])
                eng = nc.gpsimd if NORM_ENG == "pool" else nc.vector
                eng.tensor_mul(attnT[hh][:, q0:q0 + 1024],
                               av[0:DH, :], rB[:])

            # extras: global slot -> list of closures (one rep's stream = 128)
            def build_extras(rep):
                ex = {}
                vset_next = (rep + 1) % 2
                def add(slot, fn):
                    ex.setdefault(slot, []).append(fn)
                # pair-1 q/k proj pieces in phases 1-2 (needed by phase 5)
                for i in range(4):
                    add(2 + 8 * i, lambda sc4=i: qk_piece(1, sc4, 'k'))
                    add(6 + 8 * i, lambda sc4=i: qk_piece(1, sc4, 'q'))
                if rep + 1 < repeat:
                    # next pass: v pieces phases 3-5, pair-0 q/k phases 6-7
                    for st in range(16):
                        add(33 + 3 * st, lambda st=st: v_piece(vset_next, st))
                    for i in range(4):
                        add(82 + 8 * i, lambda sc4=i: qk_piece(0, sc4, 'k'))
                        add(86 + 8 * i, lambda sc4=i: qk_piece(0, sc4, 'q'))
                return ex

            # prologue (pass 0 inputs)
            for sc4 in range(4):
                qk_piece(0, sc4, 'k')
                qk_piece(0, sc4, 'q')
            for st in range(16):
                v_piece(0, st)

            def drain_one(pending, rep):
                av, vset, p, hl, qh, kt, et, ph = pending.pop(0)
                emit_av(av, vset, p, hl, kt, et, kt == 0, kt == 15)
                if kt == 15:
                    emit_norm(av, p, hl, qh)
                    if ph == 5:
                        out_proj(0, rep)
                    elif ph == 7:
                        out_proj(1, rep)

            for rep in range(repeat):
                vset = rep % 2
                extras = build_extras(rep)
                pending = []
                av = None
                for slot in range(128):
                    ph, kt = divmod(slot, 16)
                    p, hl, qh = phases[ph]
                    if kt == 0:
                        av = avp.tile([66, 1024], f32, tag="av", name="av")
                    sc = emit_scores(p, hl, qh, kt)
                    et = emit_exp(sc, ph, kt)
                    pending.append((av, vset, p, hl, qh, kt, et, ph))
                    while len(pending) > av_delay:
                        drain_one(pending, rep)
                    for fn in extras.pop(slot, ()):
                        fn()
                while pending:
                    drain_one(pending, rep)

    nc.finalize()
    return nc


def _get_nc(repeat=1):
    key = (repeat, os.environ.get("KPOLY", "0"), os.environ.get("KSCBUFS", "3"),
           os.environ.get("KETBUFS", "10"), os.environ.get("KAVDELAY", "6"),
           os.environ.get("KPRIOR", "pe"), os.environ.get("KPOLYENG", "pool"),
           os.environ.get("KNORMENG", "dve"))
    if key not in _BUILT:
        _BUILT[key] = _build(repeat)
    return _BUILT[key]


def _make_in_maps(x, Wq, bq, Wk, bk, Wv, bv, Wo, bo, prior_weight):
    pw8i = (8.0 * float(prior_weight[0])) * np.eye(128, dtype=np.float32)
    eye = np.eye(128, dtype=np.float32)
    bf = mybir.dt.np(bf16)
    ones_row = np.ones((1, 128), np.float32)
    ones_blk = np.ones((128, 8), np.float32)
    c = C4 ** (1.0 / 3.0)
    pcoef_row = np.tile(np.array([c, C1 * c, C2 * c * c, C3 * c ** 3],
                                 np.float32), (128, 1))
    xT = [np.ascontiguousarray(x[b].T) for b in range(B)]
    in_maps = []
    for c in range(N_CORES):
        b, half = c // 2, c % 2
        cs = slice(half * W_COLS, (half + 1) * W_COLS)
        in_maps.append({
            "xT": xT[b],
            "wq": np.ascontiguousarray(Wq[:, cs]),
            "wk": np.ascontiguousarray(Wk[:, cs]),
            "wv": np.ascontiguousarray(Wv[:, cs]),
            "wo": np.ascontiguousarray(Wo[cs, :]),
            "bq": np.ascontiguousarray(bq[cs].reshape(2, 128).T),
            "bk": np.ascontiguousarray(bk[cs].reshape(2, 128).T),
            "bv": np.ascontiguousarray(bv[cs].reshape(1, W_COLS)),
            "pw8i": pw8i.astype(bf),
            "eye": eye.astype(bf),
            "pcoef": pcoef_row,
            "pw8f": pw8i,
            "ones_row": ones_row,
            "ones_blk": ones_blk,
        })
    return in_maps


def run(inputs, trace=False, trace_cores=None):
    """Execute on 8 cores; returns (output [B,S,DH] f32, BassKernelResults)."""
    args = {k: np.asarray(v) for k, v in inputs.items()}
    nc = _get_nc()
    in_maps = _make_in_maps(
        args["x"], args["Wq"], args["bq"], args["Wk"], args["bk"],
        args["Wv"], args["bv"], args["Wo"], args["bo"], args["prior_weight"])
    res = run_bass_kernel_spmd(
        nc, in_maps, list(range(N_CORES)), trace=trace,
        **({"trace_cores": trace_cores} if trace_cores else {}))
    bo = args["bo"].astype(np.float32)
    out = np.empty((B, S, DH), np.float32)
    for b in range(B):
        acc = res.results[2 * b]["outT"] + res.results[2 * b + 1]["outT"]
        out[b] = acc.T + bo
    return out, res


def kernel(**inputs) -> np.ndarray:
    out, _ = run(inputs, trace=False)
    return out
